# revision 48
# baseline (speedup 1.0000x reference)
"""Causal self-attention (B=2, T=2048, C=1024, H=16) on 8 TRN2 NeuronCores.

Sharding: data-parallel on batch (2) x tensor-parallel on heads (4 groups of
4 heads) = 8 cores. Each core computes, for its batch b and head group g:
  QKV^T projection for its 256 qkv columns, causal flash-style attention for
  its 4 heads, and a partial output projection  Y_g @ W_proj[256g:256(g+1)].
The host sums the 4 partial projections per batch and adds
b_eff = b_proj + b_v @ W_proj (the V bias commutes through softmax-weighted
averaging, so it is folded into the output bias instead of being added to V
on-chip).

On-chip dataflow (bf16 matmuls), per 512-query block:
  Q^T  = Wq.T @ x via lhsT=Wq chunks   [qcol-part, tq]   (+bias on ACT)
  K^T  likewise; V natural             [tk-part, vcol]
  per key block tk (the pipeline unit, both heads of the pair):
    S^T  = K^T_tk.T @ Q^T              [tk-part, tq]  the two heads run
                                       row-tiled (PE 64-row tiles, concurrent)
    E    = exp(S^T/8)    (ACT, PSUM->SBUF, subrange only)
    tri-mask multiply on the 128-wide diagonal strip (DVE)
    Ynum^T, denom += [V_h | 1].T @ E   [65-part, tq]  (PSUM accumulated)
  The unit loop is software-pipelined (S/exp issued 2 units ahead of AV).
  Y^T  = Ynum^T * recip(denom)         (DVE recip + GPSIMD bcast + DVE mul)
  out += Y^T.T @ Wp  partial projections interleaved into the NEXT query
  block's QKV phase; final 8 emitted at the end with copies split across
  ACT/DVE; partials leave as bf16 to halve output DMA.
A short burst of dummy matmuls runs during the initial DMA wait so the PE
HAM clock-gate is already released when real matmuls start.
"""

import ml_dtypes
import numpy as np

import concourse.bacc as bacc
import concourse.mybir as mybir
from concourse import bass_utils
from concourse.bass import ts
from concourse.tile import TileContext

P = 128
T = 2048
C = 1024
KO = C // P          # 8 contraction chunks over C
HC = 256             # qkv columns per core (4 heads x 64)
NH = 4               # heads per core
D = 64
NTK = T // P         # 16 key blocks
TQB = 512            # query block (free dim)
NQ = T // TQB        # 4 query blocks
SCALE = 1.0 / np.sqrt(D)

f32 = mybir.dt.float32
bf16 = mybir.dt.bfloat16
AF = mybir.ActivationFunctionType
ALU = mybir.AluOpType

_NC = None


def _build():
    nc = bacc.Bacc(trn_type="TRN2", target_bir_lowering=False, debug=False)

    xt_d = nc.dram_tensor("xt", [C, T], bf16, kind="ExternalInput")
    wq_d = nc.dram_tensor("wq", [C, HC], bf16, kind="ExternalInput")
    wk_d = nc.dram_tensor("wk", [C, HC], bf16, kind="ExternalInput")
    wv_d = nc.dram_tensor("wv", [C, HC], bf16, kind="ExternalInput")
    wp_d = nc.dram_tensor("wp", [HC, C], bf16, kind="ExternalInput")
    bqk_d = nc.dram_tensor("bqk", [P, 4], f32, kind="ExternalInput")
    out_d = nc.dram_tensor("out", [T, C], bf16, kind="ExternalOutput")

    with TileContext(nc) as tc:
        with (
            tc.tile_pool(name="persist", bufs=1) as pp,
            tc.tile_pool(name="ps", bufs=2, space="PSUM") as ps,
            tc.tile_pool(name="psy", bufs=2, space="PSUM") as psy,
            tc.tile_pool(name="pjp", bufs=2, space="PSUM") as pjp,
            tc.tile_pool(name="xs", bufs=4) as xsp,
            tc.tile_pool(name="e", bufs=8) as ep,
            tc.tile_pool(name="r", bufs=8) as rp,
            tc.tile_pool(name="o", bufs=6) as op,
        ):
            wq = pp.tile([P, KO, HC], bf16, tag="wq")
            wk = pp.tile([P, KO, HC], bf16, tag="wk")
            wv = pp.tile([P, KO, HC], bf16, tag="wv")
            wp = pp.tile([P, 2, C], bf16, tag="wp")
            bqk = pp.tile([P, 4], f32, tag="bqk")
            qt = pp.tile([P, 2, T], bf16, tag="qt")
            kt = pp.tile([P, 2, T], bf16, tag="kt")
            v = pp.tile([P, NTK, NH, D + 1], bf16, tag="v")
            yt = pp.tile([P, 2, T], bf16, tag="yt")
            tri = pp.tile([P, 2, P], bf16, tag="tri")
            wrm = pp.tile([P, TQB], bf16, tag="wrm")

            wq_ap = wq_d.ap().rearrange("(ko p) n -> p ko n", p=P)
            wk_ap = wk_d.ap().rearrange("(ko p) n -> p ko n", p=P)
            wv_ap = wv_d.ap().rearrange("(ko p) n -> p ko n", p=P)
            wp_ap = wp_d.ap().rearrange("(kc p) n -> p kc n", p=P)
            xt_ap = xt_d.ap().rearrange("(ko p) t -> p ko t", p=P)

            xs_t = [
                xsp.tile([P, KO, TQB], bf16, tag="xs", name=f"xs{i}")
                for i in range(NQ)
            ]
            # warm the ACT function table before it's on the critical path
            warm = rp.tile([1, 1], f32, tag="warm")
            nc.vector.memset(warm[:], 0.0)
            nc.scalar.activation(warm[:], warm[:], AF.Exp)
            nc.vector.memset(wrm[:], 0.0)

            # DMA priority order on the sync queue: first query block's first
            # chunks, then coarse transfers ordered by first use (each
            # dma_start costs ~0.6us of issue time on its queue).
            nc.sync.dma_start(xs_t[0][:, 0:2, :], xt_ap[:, 0:2, ts(0, TQB)])
            nc.sync.dma_start(wq[:, 0:2, :], wq_ap[:, 0:2, :])
            nc.sync.dma_start(xs_t[0][:, 2:KO, :], xt_ap[:, 2:KO, ts(0, TQB)])
            nc.sync.dma_start(wq[:, 2:KO, :], wq_ap[:, 2:KO, :])
            nc.sync.dma_start(wk[:, 0:4, :], wk_ap[:, 0:4, :])
            nc.sync.dma_start(wk[:, 4:KO, :], wk_ap[:, 4:KO, :])
            # the bias transfer is tiny but needs 128 descriptors (~3.4us of
            # issue time) -- park it on the otherwise-idle scalar queue so it
            # does not stall the x/weight stream on the sync queue
            nc.scalar.dma_start(bqk[:], bqk_d.ap())
            nc.sync.dma_start(wv[:], wv_ap[:])
            nc.sync.dma_start(xs_t[1][:], xt_ap[:, :, ts(1, TQB)])
            nc.sync.dma_start(wp[:], wp_ap[:])
            for tqb in range(2, NQ):
                nc.sync.dma_start(xs_t[tqb][:], xt_ap[:, :, ts(tqb, TQB)])

            # HAM warm-up: keep the PE busy with dummy matmuls while the
            # first input DMAs stream in, so the clock gate is released
            # (K=8/8) by the time real matmuls start.
            wpsum = pjp.tile([P, TQB], f32, tag="pj", name="wmm")

            def warm_fill(n):
                for _ in range(n):
                    nc.tensor.matmul(
                        wpsum[:], wrm[:, 0:P], wrm[:], start=True, stop=True
                    )

            warm_fill(9)

            # ones column for the denominator row of V_aug
            nc.gpsimd.memset(v[:, :, :, D : D + 1], 1.0)
            # triangle mask tri[p, j, f] = 1 iff p <= f (dup'd on dim1 so a
            # single DVE op masks both heads of a unit)
            nc.gpsimd.memset(tri[:], 1.0)
            for j in range(2):
                nc.gpsimd.affine_select(
                    out=tri[:, j, :],
                    in_=tri[:, j, :],
                    compare_op=ALU.is_ge,
                    fill=0.0,
                    base=0,
                    pattern=[[1, P]],
                    channel_multiplier=-1,
                )

            ot_tiles = {}
            pending = []

            def emit_proj(mt, nb, tail=False):
                # non-tail drains run during the QKV phase where the psy
                # (attention-output) banks are idle -- allocating from psy
                # keeps the pq/pk/pv rotation in pjp free of drain readers
                # at the tail both the QKV (pjp) and attention-score (ps)
                # pools are idle: alternate between them for 4 buffers of
                # projection double-buffering
                if tail:
                    pool, tag = (pjp, "pj") if (2 * mt + nb) % 2 == 0 else (ps, "s")
                else:
                    pool, tag = psy, "y"
                pj = pool.tile([P, 512], f32, tag=tag, name="pj")
                for kc in range(2):
                    nc.tensor.matmul(
                        pj[:],
                        yt[:, kc, ts(mt, P)],
                        wp[:, kc, ts(nb, 512)],
                        start=(kc == 0),
                        stop=(kc == 1),
                    )
                key = mt
                if key not in ot_tiles:
                    ot_tiles[key] = op.tile([P, C], bf16, tag="ot", name=f"ot{mt}")
                ot = ot_tiles[key]
                if tail:
                    # no exps remain at the tail: alternate the PSUM
                    # evacuation between ACT and DVE and DMA each 512-column
                    # half independently to shorten the end-of-kernel chain
                    if nb == 0:
                        nc.scalar.copy(ot[:, ts(nb, 512)], pj[:])
                    else:
                        nc.vector.tensor_copy(ot[:, ts(nb, 512)], pj[:])
                    nc.sync.dma_start(
                        out_d.ap()[ts(mt, P), ts(nb, 512)], ot[:, ts(nb, 512)]
                    )
                else:
                    # drains run in the QKV phase where ACT is nearly idle:
                    # alternate the evacuations so DVE keeps its slack for
                    # the normalize chains at block boundaries; DMA each
                    # half as soon as its copy lands so the ot buffer and
                    # the sync queue drain earlier
                    if nb == 0:
                        nc.scalar.copy(ot[:, ts(nb, 512)], pj[:])
                    else:
                        nc.vector.tensor_copy(ot[:, ts(nb, 512)], pj[:])
                    nc.sync.dma_start(
                        out_d.ap()[ts(mt, P), ts(nb, 512)], ot[:, ts(nb, 512)]
                    )

            def drain(n):
                for _ in range(n):
                    if pending:
                        emit_proj(*pending.pop(0))

            for tqb in range(NQ):
                xs = xs_t[tqb]
                ntk = 4 * (tqb + 1)

                # -- Q^T / K^T for this query block (bias add on ACT) --
                for cc in range(2):
                    pq = pjp.tile([P, TQB], f32, tag="pj", name="pq")
                    for ko in range(KO):
                        nc.tensor.matmul(
                            pq[:],
                            wq[:, ko, ts(cc, P)],
                            xs[:, ko, :],
                            start=(ko == 0),
                            stop=(ko == KO - 1),
                        )
                        if tqb == 0 and cc == 0 and ko == 1:
                            # keep the PE HAM-busy while the rest of the
                            # first x block streams in (the dummies write a
                            # different PSUM bank, so the open accumulation
                            # group is unaffected)
                            warm_fill(12)
                    nc.scalar.activation(
                        qt[:, cc, ts(tqb, TQB)],
                        pq[:],
                        AF.Identity,
                        bias=bqk[:, cc : cc + 1],
                    )
                    if tqb > 0 and cc == 1:
                        drain(1)
                    pk = pjp.tile([P, TQB], f32, tag="pj", name="pk")
                    for ko in range(KO):
                        nc.tensor.matmul(
                            pk[:],
                            wk[:, ko, ts(cc, P)],
                            xs[:, ko, :],
                            start=(ko == 0),
                            stop=(ko == KO - 1),
                        )
                    nc.scalar.activation(
                        kt[:, cc, ts(tqb, TQB)],
                        pk[:],
                        AF.Identity,
                        bias=bqk[:, 2 + cc : 3 + cc],
                    )
                    if tqb > 0:
                        drain(1)

                # pre-issue the first two S units of a head pair so their
                # exps cook on ACT underneath other PE work and the AV
                # stream can start the moment that pair's loop opens
                def pre_issue(hp_, tk0, lst):
                    jd0 = tk0 - 4 * tqb
                    lo0 = jd0 * P if jd0 > 0 else 0
                    s_pre = ps.tile([P, 2, TQB], f32, tag="s", name="s")
                    e_pre = ep.tile([P, 2, TQB], bf16, tag="e", name="e")
                    nc.tensor.matmul(
                        s_pre[:, 0, lo0:],
                        kt[0:64, hp_, ts(tk0, P)],
                        qt[0:64, hp_, ts(tqb, TQB)][:, lo0:],
                        start=True, stop=True,
                    )
                    nc.tensor.matmul(
                        s_pre[:, 1, lo0:],
                        kt[64:128, hp_, ts(tk0, P)],
                        qt[64:128, hp_, ts(tqb, TQB)][:, lo0:],
                        start=True, stop=True,
                    )
                    nc.scalar.activation(
                        e_pre[:, :, lo0:], s_pre[:, :, lo0:], AF.Exp,
                        scale=SCALE,
                    )
                    if jd0 >= 0:
                        nc.vector.tensor_tensor(
                            e_pre[:, :, lo0 : lo0 + P],
                            e_pre[:, :, lo0 : lo0 + P],
                            tri[:],
                            ALU.mult,
                        )
                    lst.append((e_pre, lo0))

                pre_e = []
                pre_issue(0, 0, pre_e)
                pre_issue(0, 1, pre_e)
                pre_e1 = []

                # -- V for the 4 key blocks of this slice (no bias: folded
                # into the host-side output bias) --
                for lt in range(4):
                    tk = 4 * tqb + lt
                    pv = pjp.tile([P, TQB], f32, tag="pj", name="pv")
                    for ko in range(KO):
                        nc.tensor.matmul(
                            pv[:, :HC],
                            xs[:, ko, ts(lt, P)],
                            wv[:, ko, :],
                            start=(ko == 0),
                            stop=(ko == KO - 1),
                        )
                    nc.vector.tensor_copy(
                        v[:, tk, :, 0:D],
                        pv[:, :HC].rearrange("p (h a) -> p h a", h=NH),
                    )
                    if tqb > 0 and lt == 1:
                        drain(1)

                # -- causal attention, both head pairs, software-pipelined --
                for hp in range(2):
                    h0a, h1a = 2 * hp, 2 * hp + 1
                    y0 = psy.tile([D + 1, TQB], f32, tag="y", name="y0")
                    y1 = psy.tile([D + 1, TQB], f32, tag="y", name="y1")
                    q0 = qt[0:64, hp, ts(tqb, TQB)]
                    q1 = qt[64:128, hp, ts(tqb, TQB)]

                    unit_e = []  # (e_tile, lo) per issued unit

                    def issue_s(tk):
                        jd = tk - 4 * tqb
                        lo = jd * P if jd > 0 else 0
                        s = ps.tile([P, 2, TQB], f32, tag="s", name="s")
                        e = ep.tile([P, 2, TQB], bf16, tag="e", name="e")
                        nc.tensor.matmul(
                            s[:, 0, lo:], kt[0:64, hp, ts(tk, P)], q0[:, lo:],
                            start=True, stop=True,
                        )
                        nc.tensor.matmul(
                            s[:, 1, lo:], kt[64:128, hp, ts(tk, P)], q1[:, lo:],
                            start=True, stop=True,
                        )
                        nc.scalar.activation(
                            e[:, :, lo:], s[:, :, lo:], AF.Exp, scale=SCALE
                        )
                        if jd >= 0:
                            nc.vector.tensor_tensor(
                                e[:, :, lo : lo + P],
                                e[:, :, lo : lo + P],
                                tri[:],
                                ALU.mult,
                            )
                        unit_e.append((e, lo))

                    def issue_av(tk):
                        e, lo = unit_e[tk]
                        nc.tensor.matmul(
                            y0[:, lo:], v[:, tk, h0a, :], e[:, 0, lo:],
                            start=(tk == 0), stop=(tk == ntk - 1),
                            skip_group_check=True,
                        )
                        nc.tensor.matmul(
                            y1[:, lo:], v[:, tk, h1a, :], e[:, 1, lo:],
                            start=(tk == 0), stop=(tk == ntk - 1),
                            skip_group_check=True,
                        )

                    def normalize(lo_col, w):
                        # columns [lo_col, lo_col+w) of the accumulators are
                        # final once every AV unit whose causal window covers
                        # them has landed
                        hs = slice(lo_col, lo_col + w)
                        c0 = tqb * TQB + lo_col
                        for yi, ya in enumerate((y0, y1)):
                            dn = rp.tile([1, w], f32, tag="den", name="dn")
                            nc.vector.tensor_copy(dn[:], ya[64:65, hs])
                            rc = rp.tile([1, w], f32, tag="rec", name="rc")
                            nc.vector.reciprocal_approx_fast(rc[:], dn[:])
                            rb = rp.tile([D, w], f32, tag="rb", name="rb")
                            nc.gpsimd.partition_broadcast(rb[:], rc[:])
                            nc.vector.tensor_tensor(
                                yt[yi * 64 : yi * 64 + 64, hp, c0 : c0 + w],
                                ya[0:64, hs],
                                rb[:],
                                ALU.mult,
                            )

                    last = tqb == NQ - 1 and hp == 1
                    if hp == 0:
                        # units 0/1 were pre-issued before the V phase
                        unit_e.extend(pre_e)
                        issue_av(0)
                        issue_av(1)
                        for tk in range(2, ntk):
                            issue_s(tk)
                            # spread drains into the late, exp-paced units
                            # so the PE always has filler while ACT catches
                            # up (the lag grows ~180ns per unit)
                            if tk in (3, 7, 11):
                                drain(1)
                            if tk >= 4:
                                issue_av(tk - 2)
                        # pre-issue head-pair 1's first units: their exps
                        # run under this pair's tail AVs and normalizes
                        pre_issue(1, 0, pre_e1)
                        pre_issue(1, 1, pre_e1)
                    else:
                        unit_e.extend(pre_e1)
                        drain(1)
                        issue_av(0)
                        issue_av(1)
                        for tk in range(2, ntk):
                            issue_s(tk)
                            if tk in (3, 5, 7, 9):
                                drain(1)
                            if tk >= 4:
                                issue_av(tk - 2)
                    if not last:
                        normalize(0, 256)
                        issue_av(ntk - 2)
                        issue_av(ntk - 1)
                        normalize(256, 256)
                    else:
                        # last head pair: the final two AV units go first so
                        # the in-order PE never queues them behind
                        # normalize-gated projections
                        normalize(0, 256)
                        issue_av(ntk - 2)
                        issue_av(ntk - 1)
                        emit_proj(4 * tqb + 0, 0, tail=True)
                        emit_proj(4 * tqb + 0, 1, tail=True)
                        emit_proj(4 * tqb + 1, 0, tail=True)
                        emit_proj(4 * tqb + 1, 1, tail=True)
                        normalize(256, 256)
                        emit_proj(4 * tqb + 2, 0, tail=True)
                        emit_proj(4 * tqb + 2, 1, tail=True)
                        emit_proj(4 * tqb + 3, 0, tail=True)
                        emit_proj(4 * tqb + 3, 1, tail=True)

                if tqb < NQ - 1:
                    pending += [
                        (mt, nb)
                        for mt in range(4 * tqb, 4 * tqb + 4)
                        for nb in range(2)
                    ]
                else:
                    # last block: heads 0-1 (hp 0) projections still pending;
                    # hp 1 was flushed inline above
                    pass

            for mt_nb in pending:
                emit_proj(*mt_nb, tail=True)

    nc.compile()
    return nc


def _get_nc():
    global _NC
    if _NC is None:
        _NC = _build()
    return _NC


def _shard(x, W_qkv, b_qkv, W_proj, b_proj):
    x = np.ascontiguousarray(np.asarray(x, dtype=np.float32))
    W_qkv = np.ascontiguousarray(np.asarray(W_qkv, dtype=np.float32))
    b_qkv = np.ascontiguousarray(np.asarray(b_qkv, dtype=np.float32))
    W_proj = np.ascontiguousarray(np.asarray(W_proj, dtype=np.float32))
    in_maps = []
    for core in range(8):
        b, g = core // 4, core % 4
        cs = slice(g * HC, (g + 1) * HC)
        bq = b_qkv[0 * C :][cs]
        bk = b_qkv[1 * C :][cs]
        bqk = np.stack(
            [bq[0:P], bq[P : 2 * P], bk[0:P], bk[P : 2 * P]], axis=1
        )
        in_maps.append(
            {
                "xt": np.ascontiguousarray(x[b].T.astype(ml_dtypes.bfloat16)),
                "wq": np.ascontiguousarray(
                    W_qkv[:, 0 * C :][:, cs].astype(ml_dtypes.bfloat16)
                ),
                "wk": np.ascontiguousarray(
                    W_qkv[:, 1 * C :][:, cs].astype(ml_dtypes.bfloat16)
                ),
                "wv": np.ascontiguousarray(
                    W_qkv[:, 2 * C :][:, cs].astype(ml_dtypes.bfloat16)
                ),
                "wp": np.ascontiguousarray(W_proj[cs, :].astype(ml_dtypes.bfloat16)),
                "bqk": np.ascontiguousarray(bqk),
            }
        )
    return in_maps


def _gather(results, b_qkv, W_proj, b_proj):
    b_qkv = np.asarray(b_qkv, dtype=np.float32)
    W_proj = np.asarray(W_proj, dtype=np.float32)
    b_eff = np.asarray(b_proj, dtype=np.float32) + b_qkv[2 * C :] @ W_proj
    y = np.empty((2, T, C), dtype=np.float32)
    for b in range(2):
        acc = results[4 * b]["out"].astype(np.float32)
        for g in range(1, 4):
            acc = acc + results[4 * b + g]["out"].astype(np.float32)
        y[b] = acc + b_eff
    return y


def kernel(x, W_qkv, b_qkv, W_proj, b_proj):
    nc = _get_nc()
    in_maps = _shard(x, W_qkv, b_qkv, W_proj, b_proj)
    res = bass_utils.run_bass_kernel_spmd(nc, in_maps, core_ids=list(range(8)))
    return _gather(res.results, b_qkv, W_proj, b_proj)


# revision 50
# speedup vs baseline: 1.0207x; 1.0207x over previous
"""Causal self-attention (B=2, T=2048, C=1024, H=16) on 8 TRN2 NeuronCores.

Sharding: data-parallel on batch (2) x tensor-parallel on heads (4 groups of
4 heads) = 8 cores. Each core computes, for its batch b and head group g:
  QKV^T projection for its 256 qkv columns, causal flash-style attention for
  its 4 heads, and a partial output projection  Y_g @ W_proj[256g:256(g+1)].
The host sums the 4 partial projections per batch and adds
b_eff = b_proj + b_v @ W_proj (the V bias commutes through softmax-weighted
averaging, so it is folded into the output bias instead of being added to V
on-chip).

On-chip dataflow (bf16 matmuls), per 512-query block:
  Q^T  = Wq.T @ x via lhsT=Wq chunks   [qcol-part, tq]   (+bias on ACT)
  K^T  likewise; V natural             [tk-part, vcol]
  per key block tk (the pipeline unit, both heads of the pair):
    S^T  = K^T_tk.T @ Q^T              [tk-part, tq]  the two heads run
                                       row-tiled (PE 64-row tiles, concurrent)
    E    = exp(S^T/8)    (ACT, PSUM->SBUF, subrange only)
    tri-mask multiply on the 128-wide diagonal strip (DVE)
    Ynum^T, denom += [V_h | 1].T @ E   [65-part, tq]  (PSUM accumulated)
  The unit loop is software-pipelined (S/exp issued 2 units ahead of AV).
  Y^T  = Ynum^T * recip(denom)         (DVE recip + GPSIMD bcast + DVE mul)
  out += Y^T.T @ Wp  partial projections interleaved into the NEXT query
  block's QKV phase; final 8 emitted at the end with copies split across
  ACT/DVE; partials leave as bf16 to halve output DMA.
A short burst of dummy matmuls runs during the initial DMA wait so the PE
HAM clock-gate is already released when real matmuls start.
"""

import ml_dtypes
import numpy as np

import concourse.bacc as bacc
import concourse.mybir as mybir
from concourse import bass_utils
from concourse.bass import ts
from concourse.tile import TileContext

P = 128
T = 2048
C = 1024
KO = C // P          # 8 contraction chunks over C
HC = 256             # qkv columns per core (4 heads x 64)
NH = 4               # heads per core
D = 64
NTK = T // P         # 16 key blocks
TQB = 512            # query block (free dim)
NQ = T // TQB        # 4 query blocks
SCALE = 1.0 / np.sqrt(D)

f32 = mybir.dt.float32
bf16 = mybir.dt.bfloat16
AF = mybir.ActivationFunctionType
ALU = mybir.AluOpType

_NC = None


def _build():
    nc = bacc.Bacc(trn_type="TRN2", target_bir_lowering=False, debug=False)

    xt_d = nc.dram_tensor("xt", [C, T], bf16, kind="ExternalInput")
    wq_d = nc.dram_tensor("wq", [C, HC], bf16, kind="ExternalInput")
    wk_d = nc.dram_tensor("wk", [C, HC], bf16, kind="ExternalInput")
    wv_d = nc.dram_tensor("wv", [C, HC], bf16, kind="ExternalInput")
    wp_d = nc.dram_tensor("wp", [HC, C], bf16, kind="ExternalInput")
    bqk_d = nc.dram_tensor("bqk", [P, 4], f32, kind="ExternalInput")
    out_d = nc.dram_tensor("out", [T, C], bf16, kind="ExternalOutput")

    with TileContext(nc) as tc:
        with (
            tc.tile_pool(name="persist", bufs=1) as pp,
            tc.tile_pool(name="ps", bufs=2, space="PSUM") as ps,
            tc.tile_pool(name="psy", bufs=2, space="PSUM") as psy,
            tc.tile_pool(name="pjp", bufs=2, space="PSUM") as pjp,
            tc.tile_pool(name="xs", bufs=4) as xsp,
            tc.tile_pool(name="e", bufs=8) as ep,
            tc.tile_pool(name="r", bufs=8) as rp,
            tc.tile_pool(name="o", bufs=6) as op,
        ):
            wq = pp.tile([P, KO, HC], bf16, tag="wq")
            wk = pp.tile([P, KO, HC], bf16, tag="wk")
            wv = pp.tile([P, KO, HC], bf16, tag="wv")
            wp = pp.tile([P, 2, C], bf16, tag="wp")
            bqk = pp.tile([P, 4], f32, tag="bqk")
            qt = pp.tile([P, 2, T], bf16, tag="qt")
            kt = pp.tile([P, 2, T], bf16, tag="kt")
            v = pp.tile([P, NTK, NH, D + 1], bf16, tag="v")
            yt = pp.tile([P, 2, T], bf16, tag="yt")
            tri = pp.tile([P, 2, P], bf16, tag="tri")
            wrm = pp.tile([P, TQB], bf16, tag="wrm")

            wq_ap = wq_d.ap().rearrange("(ko p) n -> p ko n", p=P)
            wk_ap = wk_d.ap().rearrange("(ko p) n -> p ko n", p=P)
            wv_ap = wv_d.ap().rearrange("(ko p) n -> p ko n", p=P)
            wp_ap = wp_d.ap().rearrange("(kc p) n -> p kc n", p=P)
            xt_ap = xt_d.ap().rearrange("(ko p) t -> p ko t", p=P)

            xs_t = [
                xsp.tile([P, KO, TQB], bf16, tag="xs", name=f"xs{i}")
                for i in range(NQ)
            ]
            # warm the ACT function table before it's on the critical path
            warm = rp.tile([1, 1], f32, tag="warm")
            nc.vector.memset(warm[:], 0.0)
            nc.scalar.activation(warm[:], warm[:], AF.Exp)
            nc.vector.memset(wrm[:], 0.0)

            # DMA priority order on the sync queue: first query block's first
            # chunks, then coarse transfers ordered by first use (each
            # dma_start costs ~0.6us of issue time on its queue).
            nc.sync.dma_start(xs_t[0][:, 0:2, :], xt_ap[:, 0:2, ts(0, TQB)])
            nc.sync.dma_start(wq[:, 0:2, :], wq_ap[:, 0:2, :])
            nc.sync.dma_start(xs_t[0][:, 2:KO, :], xt_ap[:, 2:KO, ts(0, TQB)])
            nc.sync.dma_start(wq[:, 2:KO, :], wq_ap[:, 2:KO, :])
            nc.sync.dma_start(wk[:, 0:4, :], wk_ap[:, 0:4, :])
            nc.sync.dma_start(wk[:, 4:KO, :], wk_ap[:, 4:KO, :])
            # the bias transfer is tiny but needs 128 descriptors (~3.4us of
            # issue time) -- park it on the otherwise-idle scalar queue so it
            # does not stall the x/weight stream on the sync queue
            nc.scalar.dma_start(bqk[:], bqk_d.ap())
            nc.sync.dma_start(wv[:], wv_ap[:])
            nc.sync.dma_start(xs_t[1][:], xt_ap[:, :, ts(1, TQB)])
            nc.sync.dma_start(wp[:], wp_ap[:])
            for tqb in range(2, NQ):
                nc.sync.dma_start(xs_t[tqb][:], xt_ap[:, :, ts(tqb, TQB)])

            # HAM warm-up: keep the PE busy with dummy matmuls while the
            # first input DMAs stream in, so the clock gate is released
            # (K=8/8) by the time real matmuls start.
            wpsum = pjp.tile([P, TQB], f32, tag="pj", name="wmm")

            def warm_fill(n):
                for _ in range(n):
                    nc.tensor.matmul(
                        wpsum[:], wrm[:, 0:P], wrm[:], start=True, stop=True
                    )

            warm_fill(9)

            # ones column for the denominator row of V_aug
            nc.gpsimd.memset(v[:, :, :, D : D + 1], 1.0)
            # triangle mask tri[p, j, f] = 1 iff p <= f (dup'd on dim1 so a
            # single DVE op masks both heads of a unit)
            nc.gpsimd.memset(tri[:], 1.0)
            for j in range(2):
                nc.gpsimd.affine_select(
                    out=tri[:, j, :],
                    in_=tri[:, j, :],
                    compare_op=ALU.is_ge,
                    fill=0.0,
                    base=0,
                    pattern=[[1, P]],
                    channel_multiplier=-1,
                )

            ot_tiles = {}
            pending = []

            def emit_proj(mt, nb, tail=False):
                # non-tail drains run during the QKV phase where the psy
                # (attention-output) banks are idle -- allocating from psy
                # keeps the pq/pk/pv rotation in pjp free of drain readers
                # at the tail both the QKV (pjp) and attention-score (ps)
                # pools are idle: alternate between them for 4 buffers of
                # projection double-buffering
                if tail:
                    pool, tag = (pjp, "pj") if (2 * mt + nb) % 2 == 0 else (ps, "s")
                else:
                    pool, tag = psy, "y"
                pj = pool.tile([P, 512], f32, tag=tag, name="pj")
                for kc in range(2):
                    nc.tensor.matmul(
                        pj[:],
                        yt[:, kc, ts(mt, P)],
                        wp[:, kc, ts(nb, 512)],
                        start=(kc == 0),
                        stop=(kc == 1),
                    )
                key = mt
                if key not in ot_tiles:
                    ot_tiles[key] = op.tile([P, C], bf16, tag="ot", name=f"ot{mt}")
                ot = ot_tiles[key]
                if tail:
                    # no exps remain at the tail: alternate the PSUM
                    # evacuation between ACT and DVE and DMA each 512-column
                    # half independently to shorten the end-of-kernel chain
                    if nb == 0:
                        nc.scalar.copy(ot[:, ts(nb, 512)], pj[:])
                    else:
                        nc.vector.tensor_copy(ot[:, ts(nb, 512)], pj[:])
                    nc.sync.dma_start(
                        out_d.ap()[ts(mt, P), ts(nb, 512)], ot[:, ts(nb, 512)]
                    )
                else:
                    # drains run in the QKV phase where ACT is nearly idle:
                    # alternate the evacuations so DVE keeps its slack for
                    # the normalize chains at block boundaries
                    if nb == 0:
                        nc.scalar.copy(ot[:, ts(nb, 512)], pj[:])
                    else:
                        nc.vector.tensor_copy(ot[:, ts(nb, 512)], pj[:])
                    if nb == 1:
                        nc.sync.dma_start(out_d.ap()[ts(mt, P), :], ot[:])

            def drain(n):
                for _ in range(n):
                    if pending:
                        emit_proj(*pending.pop(0))

            for tqb in range(NQ):
                xs = xs_t[tqb]
                ntk = 4 * (tqb + 1)

                # -- Q^T / K^T for this query block (bias add on ACT) --
                for cc in range(2):
                    pq = pjp.tile([P, TQB], f32, tag="pj", name="pq")
                    for ko in range(KO):
                        nc.tensor.matmul(
                            pq[:],
                            wq[:, ko, ts(cc, P)],
                            xs[:, ko, :],
                            start=(ko == 0),
                            stop=(ko == KO - 1),
                        )
                        if tqb == 0 and cc == 0 and ko == 1:
                            # keep the PE HAM-busy while the rest of the
                            # first x block streams in (the dummies write a
                            # different PSUM bank, so the open accumulation
                            # group is unaffected)
                            warm_fill(12)
                    nc.scalar.activation(
                        qt[:, cc, ts(tqb, TQB)],
                        pq[:],
                        AF.Identity,
                        bias=bqk[:, cc : cc + 1],
                    )
                    if tqb > 0 and cc == 1:
                        drain(1)
                    pk = pjp.tile([P, TQB], f32, tag="pj", name="pk")
                    for ko in range(KO):
                        nc.tensor.matmul(
                            pk[:],
                            wk[:, ko, ts(cc, P)],
                            xs[:, ko, :],
                            start=(ko == 0),
                            stop=(ko == KO - 1),
                        )
                    nc.scalar.activation(
                        kt[:, cc, ts(tqb, TQB)],
                        pk[:],
                        AF.Identity,
                        bias=bqk[:, 2 + cc : 3 + cc],
                    )
                    if tqb > 0:
                        drain(1)

                # pre-issue the first two S units of a head pair so their
                # exps cook on ACT underneath other PE work and the AV
                # stream can start the moment that pair's loop opens
                def pre_issue(hp_, tk0, lst):
                    jd0 = tk0 - 4 * tqb
                    lo0 = jd0 * P if jd0 > 0 else 0
                    s_pre = ps.tile([P, 2, TQB], f32, tag="s", name="s")
                    e_pre = ep.tile([P, 2, TQB], bf16, tag="e", name="e")
                    nc.tensor.matmul(
                        s_pre[:, 0, lo0:],
                        kt[0:64, hp_, ts(tk0, P)],
                        qt[0:64, hp_, ts(tqb, TQB)][:, lo0:],
                        start=True, stop=True,
                    )
                    nc.tensor.matmul(
                        s_pre[:, 1, lo0:],
                        kt[64:128, hp_, ts(tk0, P)],
                        qt[64:128, hp_, ts(tqb, TQB)][:, lo0:],
                        start=True, stop=True,
                    )
                    nc.scalar.activation(
                        e_pre[:, :, lo0:], s_pre[:, :, lo0:], AF.Exp,
                        scale=SCALE,
                    )
                    if jd0 >= 0:
                        nc.vector.tensor_tensor(
                            e_pre[:, :, lo0 : lo0 + P],
                            e_pre[:, :, lo0 : lo0 + P],
                            tri[:],
                            ALU.mult,
                        )
                    lst.append((e_pre, lo0))

                pre_e = []
                pre_issue(0, 0, pre_e)
                pre_issue(0, 1, pre_e)
                pre_e1 = []

                # -- V for the 4 key blocks of this slice (no bias: folded
                # into the host-side output bias) --
                for lt in range(4):
                    tk = 4 * tqb + lt
                    pv = pjp.tile([P, TQB], f32, tag="pj", name="pv")
                    for ko in range(KO):
                        nc.tensor.matmul(
                            pv[:, :HC],
                            xs[:, ko, ts(lt, P)],
                            wv[:, ko, :],
                            start=(ko == 0),
                            stop=(ko == KO - 1),
                        )
                    nc.vector.tensor_copy(
                        v[:, tk, :, 0:D],
                        pv[:, :HC].rearrange("p (h a) -> p h a", h=NH),
                    )
                    if tqb > 0 and lt == 1:
                        drain(1)

                # -- causal attention, both head pairs, software-pipelined --
                for hp in range(2):
                    h0a, h1a = 2 * hp, 2 * hp + 1
                    y0 = psy.tile([D + 1, TQB], f32, tag="y", name="y0")
                    y1 = psy.tile([D + 1, TQB], f32, tag="y", name="y1")
                    q0 = qt[0:64, hp, ts(tqb, TQB)]
                    q1 = qt[64:128, hp, ts(tqb, TQB)]

                    unit_e = []  # (e_tile, lo) per issued unit

                    def issue_s(tk):
                        jd = tk - 4 * tqb
                        lo = jd * P if jd > 0 else 0
                        s = ps.tile([P, 2, TQB], f32, tag="s", name="s")
                        e = ep.tile([P, 2, TQB], bf16, tag="e", name="e")
                        nc.tensor.matmul(
                            s[:, 0, lo:], kt[0:64, hp, ts(tk, P)], q0[:, lo:],
                            start=True, stop=True,
                        )
                        nc.tensor.matmul(
                            s[:, 1, lo:], kt[64:128, hp, ts(tk, P)], q1[:, lo:],
                            start=True, stop=True,
                        )
                        nc.scalar.activation(
                            e[:, :, lo:], s[:, :, lo:], AF.Exp, scale=SCALE
                        )
                        if jd >= 0:
                            nc.vector.tensor_tensor(
                                e[:, :, lo : lo + P],
                                e[:, :, lo : lo + P],
                                tri[:],
                                ALU.mult,
                            )
                        unit_e.append((e, lo))

                    def issue_av(tk):
                        e, lo = unit_e[tk]
                        nc.tensor.matmul(
                            y0[:, lo:], v[:, tk, h0a, :], e[:, 0, lo:],
                            start=(tk == 0), stop=(tk == ntk - 1),
                            skip_group_check=True,
                        )
                        nc.tensor.matmul(
                            y1[:, lo:], v[:, tk, h1a, :], e[:, 1, lo:],
                            start=(tk == 0), stop=(tk == ntk - 1),
                            skip_group_check=True,
                        )

                    def normalize(lo_col, w):
                        # columns [lo_col, lo_col+w) of the accumulators are
                        # final once every AV unit whose causal window covers
                        # them has landed
                        hs = slice(lo_col, lo_col + w)
                        c0 = tqb * TQB + lo_col
                        for yi, ya in enumerate((y0, y1)):
                            dn = rp.tile([1, w], f32, tag="den", name="dn")
                            nc.vector.tensor_copy(dn[:], ya[64:65, hs])
                            rc = rp.tile([1, w], f32, tag="rec", name="rc")
                            nc.vector.reciprocal_approx_fast(rc[:], dn[:])
                            rb = rp.tile([D, w], f32, tag="rb", name="rb")
                            nc.gpsimd.partition_broadcast(rb[:], rc[:])
                            nc.vector.tensor_tensor(
                                yt[yi * 64 : yi * 64 + 64, hp, c0 : c0 + w],
                                ya[0:64, hs],
                                rb[:],
                                ALU.mult,
                            )

                    last = tqb == NQ - 1 and hp == 1
                    if hp == 0:
                        # units 0/1 were pre-issued before the V phase
                        unit_e.extend(pre_e)
                        issue_av(0)
                        issue_av(1)
                        for tk in range(2, ntk):
                            issue_s(tk)
                            # spread drains into the later, exp-paced units
                            # too: the ACT lag grows ~180ns/unit, so the PE
                            # needs filler deep in the loop as well
                            if tk == 3:
                                drain(1)
                            elif tk == 9:
                                drain(1)
                            if tk >= 4:
                                issue_av(tk - 2)
                        # pre-issue head-pair 1's first units: their exps
                        # run under this pair's tail AVs and normalizes
                        pre_issue(1, 0, pre_e1)
                        pre_issue(1, 1, pre_e1)
                    else:
                        unit_e.extend(pre_e1)
                        drain(2)
                        issue_av(0)
                        issue_av(1)
                        for tk in range(2, ntk):
                            issue_s(tk)
                            if tk == 3:
                                drain(2)
                            elif tk == 5:
                                drain(2 if ntk == 8 else 1)
                            elif tk == 7:
                                drain(1)
                            if tk >= 4:
                                issue_av(tk - 2)
                    if not last:
                        normalize(0, 256)
                        issue_av(ntk - 2)
                        issue_av(ntk - 1)
                        normalize(256, 256)
                    else:
                        # last head pair: the final two AV units go first so
                        # the in-order PE never queues them behind
                        # normalize-gated projections
                        normalize(0, 256)
                        issue_av(ntk - 2)
                        issue_av(ntk - 1)
                        emit_proj(4 * tqb + 0, 0, tail=True)
                        emit_proj(4 * tqb + 0, 1, tail=True)
                        emit_proj(4 * tqb + 1, 0, tail=True)
                        emit_proj(4 * tqb + 1, 1, tail=True)
                        normalize(256, 256)
                        emit_proj(4 * tqb + 2, 0, tail=True)
                        emit_proj(4 * tqb + 2, 1, tail=True)
                        emit_proj(4 * tqb + 3, 0, tail=True)
                        emit_proj(4 * tqb + 3, 1, tail=True)

                if tqb < NQ - 1:
                    pending += [
                        (mt, nb)
                        for mt in range(4 * tqb, 4 * tqb + 4)
                        for nb in range(2)
                    ]
                else:
                    # last block: heads 0-1 (hp 0) projections still pending;
                    # hp 1 was flushed inline above
                    pass

            for mt_nb in pending:
                emit_proj(*mt_nb, tail=True)

    nc.compile()
    return nc


def _get_nc():
    global _NC
    if _NC is None:
        _NC = _build()
    return _NC


def _shard(x, W_qkv, b_qkv, W_proj, b_proj):
    x = np.ascontiguousarray(np.asarray(x, dtype=np.float32))
    W_qkv = np.ascontiguousarray(np.asarray(W_qkv, dtype=np.float32))
    b_qkv = np.ascontiguousarray(np.asarray(b_qkv, dtype=np.float32))
    W_proj = np.ascontiguousarray(np.asarray(W_proj, dtype=np.float32))
    in_maps = []
    for core in range(8):
        b, g = core // 4, core % 4
        cs = slice(g * HC, (g + 1) * HC)
        bq = b_qkv[0 * C :][cs]
        bk = b_qkv[1 * C :][cs]
        bqk = np.stack(
            [bq[0:P], bq[P : 2 * P], bk[0:P], bk[P : 2 * P]], axis=1
        )
        in_maps.append(
            {
                "xt": np.ascontiguousarray(x[b].T.astype(ml_dtypes.bfloat16)),
                "wq": np.ascontiguousarray(
                    W_qkv[:, 0 * C :][:, cs].astype(ml_dtypes.bfloat16)
                ),
                "wk": np.ascontiguousarray(
                    W_qkv[:, 1 * C :][:, cs].astype(ml_dtypes.bfloat16)
                ),
                "wv": np.ascontiguousarray(
                    W_qkv[:, 2 * C :][:, cs].astype(ml_dtypes.bfloat16)
                ),
                "wp": np.ascontiguousarray(W_proj[cs, :].astype(ml_dtypes.bfloat16)),
                "bqk": np.ascontiguousarray(bqk),
            }
        )
    return in_maps


def _gather(results, b_qkv, W_proj, b_proj):
    b_qkv = np.asarray(b_qkv, dtype=np.float32)
    W_proj = np.asarray(W_proj, dtype=np.float32)
    b_eff = np.asarray(b_proj, dtype=np.float32) + b_qkv[2 * C :] @ W_proj
    y = np.empty((2, T, C), dtype=np.float32)
    for b in range(2):
        acc = results[4 * b]["out"].astype(np.float32)
        for g in range(1, 4):
            acc = acc + results[4 * b + g]["out"].astype(np.float32)
        y[b] = acc + b_eff
    return y


def kernel(x, W_qkv, b_qkv, W_proj, b_proj):
    nc = _get_nc()
    in_maps = _shard(x, W_qkv, b_qkv, W_proj, b_proj)
    res = bass_utils.run_bass_kernel_spmd(nc, in_maps, core_ids=list(range(8)))
    return _gather(res.results, b_qkv, W_proj, b_proj)
